# revision 8
# baseline (speedup 1.0000x reference)
"""Trainium2 Bass kernel for nn_BasketballDetector (2-class softmax -> top-100
-> box decode -> greedy NMS), data-parallel over 8 NeuronCores (4 images each).

Self-contained: hardcodes shapes/sharding; takes full inputs, returns full outputs.

Pipeline per core (all engines, no big GPSIMD ucode ops):
  - one fused DMA loads both class planes in a [128, 2*4*1016] layout
  - diff = cls1 - cls0 on DVE (softmax is monotone in the logit difference)
  - per-image per-partition top-8 (vector.max / max_index) -> 1024 candidates
  - exact 112-ish threshold via pivot counting (PE column-sum matmuls)
  - prefix-sum + one-hot compaction (PE) -> dense survivor columns
  - compare-matrix rank sort (DVE+PE) -> descending (score-diff, pixel)
  - loc values gathered with indirect DMA (channel-last loc layout, 4B*4 rows)
  - box decode in column form, sigmoid scores on ACT
  - IoU matrix + 2 Jacobi iterations of greedy-NMS fixpoint (exact on stable point)
"""
import os, sys, types
import numpy as np

B = 32
NCORES = 8
BPC = B // NCORES          # 4 images per core
H, W = 270, 480
N = H * W                  # 129600
F = 1016                   # padded pixels per partition per image (128*1016 >= N)
NK = 100                   # output detections per image
NEG = -2.0e30

OFF_LT = 0                 # [128,128] LT[k,m] = 1 if k<m
OFF_ID = 128               # [128,128] identity
OFF_IR = 256               # [128,128] iota 0..127 along free
OFF_ONES = 384             # [128,448] ones
OFF_R8 = 832               # [128,8] 0..7
OFF_PC = 840               # [128,1] p*F
OFF_BN = 841               # [128,4] b*N
CW = 845

_CACHE = {}


def _install_hook():
    if "antenv.axon_hooks" in sys.modules:
        return
    mod = types.ModuleType("antenv.axon_hooks")
    _h = {}
    mod.set_axon_ntff_profile_hook = lambda h: _h.__setitem__("hook", h)
    mod.get_axon_ntff_profile_hook = lambda: _h.get("hook")
    try:
        import antenv
        antenv.axon_hooks = mod
    except ImportError:
        pass
    sys.modules["antenv.axon_hooks"] = mod
    try:
        from trn_agent_boot.trn_boot import _ntff_profile_via_ctypes
        hook = _ntff_profile_via_ctypes("/opt/axon/libaxon_pjrt.so")
        if hook is not None:
            mod.set_axon_ntff_profile_hook(hook)
    except Exception:
        pass


def make_consts():
    c = np.zeros((128, CW), np.float32)
    c[:, OFF_LT:OFF_LT + 128] = np.triu(np.ones((128, 128), np.float32), 1)
    c[:, OFF_ID:OFF_ID + 128] = np.eye(128, dtype=np.float32)
    c[:, OFF_IR:OFF_IR + 128] = np.arange(128, dtype=np.float32)[None, :]
    c[:, OFF_ONES:OFF_ONES + 448] = 1.0
    c[:, OFF_R8:OFF_R8 + 8] = np.arange(8, dtype=np.float32)[None, :]
    c[:, OFF_PC] = np.arange(128, dtype=np.float32) * F
    c[:, OFF_BN:OFF_BN + 4] = (np.arange(4, dtype=np.float32) * N)[None, :]
    return c


def build():
    import concourse.bass as bass
    import concourse.mybir as mybir
    import concourse.tile as tile
    import concourse.bacc as bacc

    AF = mybir.ActivationFunctionType
    AL = mybir.AluOpType

    nc = bacc.Bacc("TRN2", num_devices=NCORES, debug=False)
    cls_in = nc.dram_tensor("cls", [128, 2, BPC, F], mybir.dt.float32, kind="ExternalInput")
    loc_in = nc.dram_tensor("loc", [BPC, N, 4], mybir.dt.float32, kind="ExternalInput")
    cst_in = nc.dram_tensor("cst", [128, CW], mybir.dt.float32, kind="ExternalInput")
    det_out = nc.dram_tensor("det", [BPC, NK, 5], mybir.dt.float32, kind="ExternalOutput")
    keep_out = nc.dram_tensor("keep", [BPC, NK], mybir.dt.float32, kind="ExternalOutput")

    with tile.TileContext(nc) as tc:
        with tc.tile_pool(name="po", bufs=1) as po, \
             tc.tile_pool(name="pp", bufs=1) as pp, \
             tc.tile_pool(name="ps", bufs=1, space="PSUM") as psp:
            c01 = po.tile([128, 2 * BPC * F], mybir.dt.float32, tag="c01")
            d = po.tile([128, BPC * F], mybir.dt.float32, tag="d")
            cst = po.tile([128, CW], mybir.dt.float32, tag="cst")
            cv = po.tile([128, 32], mybir.dt.float32, tag="cv")
            ci = po.tile([128, 32], mybir.dt.uint32, tag="ci")
            pixf = po.tile([128, 32], mybir.dt.float32, tag="pixf")
            pairs = po.tile([128, 24 * BPC], mybir.dt.float32, tag="pairs")
            srt = po.tile([128, 2 * BPC], mybir.dt.float32, tag="srt")

            with nc.named_scope("load"):
                nc.sync.dma_start(out=c01[:], in_=cls_in.ap())
                nc.sync.dma_start(out=cst[:], in_=cst_in.ap())
            LT = cst[:, OFF_LT:OFF_LT + 128]
            ID = cst[:, OFF_ID:OFF_ID + 128]
            IR = cst[:, OFF_IR:OFF_IR + 128]
            R8 = cst[:, OFF_R8:OFF_R8 + 8]
            PC = cst[:, OFF_PC:OFF_PC + 1]
            BN = cst[:, OFF_BN:OFF_BN + 4]
            ONESROW = cst[0:1, OFF_ONES:OFF_ONES + 128]
            ONESCOL = cst[:, OFF_ONES:OFF_ONES + 1]

            with nc.named_scope("sub"):
                nc.vector.tensor_sub(out=d[:], in0=c01[:, BPC * F:], in1=c01[:, :BPC * F])
            with nc.named_scope("top8"):
                for b in range(BPC):
                    sl = d[:, b * F:(b + 1) * F]
                    nc.vector.max(out=cv[:, 8 * b:8 * b + 8], in_=sl)
                    nc.vector.max_index(out=ci[:, 8 * b:8 * b + 8],
                                        in_max=cv[:, 8 * b:8 * b + 8], in_values=sl)
                nc.vector.tensor_copy(out=pixf[:], in_=ci[:])
                nc.vector.tensor_scalar(out=pixf[:], in0=pixf[:], scalar1=PC,
                                        scalar2=None, op0=AL.add)
                for b in range(BPC):
                    pv = bass.AP(pairs[:].tensor, pairs[:].offset + 24 * b,
                                 [pairs[:].ap[0], [3, 8]])
                    pq = bass.AP(pairs[:].tensor, pairs[:].offset + 24 * b + 1,
                                 [pairs[:].ap[0], [3, 8]])
                    pone = bass.AP(pairs[:].tensor, pairs[:].offset + 24 * b + 2,
                                   [pairs[:].ap[0], [3, 8]])
                    nc.vector.tensor_copy(out=pv, in_=cv[:, 8 * b:8 * b + 8])
                    nc.vector.tensor_copy(out=pq, in_=pixf[:, 8 * b:8 * b + 8])
                    nc.vector.tensor_copy(out=pone, in_=bass.AP(
                        cst[:].tensor, cst[:].offset + OFF_ONES, [cst[:].ap[0], [0, 8]]))

            # ---- per-image exact top-128 sorted (val, pix) ----
            for b in range(BPC):
                with nc.named_scope(f"sort{b}"):
                    g = f"i{b}"
                    mrow = pp.tile([1, 128], mybir.dt.float32, tag=f"mrow{g}", name=f"mrow{g}")
                    Call = pp.tile([128, 1024], mybir.dt.float32, tag=f"Call{g}", name=f"Call{g}")
                    cntrow = pp.tile([1, 128], mybir.dt.float32, tag=f"cnr{g}", name=f"cnr{g}")
                    tsel = pp.tile([1, 128], mybir.dt.float32, tag=f"tsel{g}", name=f"tsel{g}")
                    tmin = pp.tile([1, 1], mybir.dt.float32, tag=f"tmin{g}", name=f"tmin{g}")
                    taub = pp.tile([128, 1], mybir.dt.float32, tag=f"taub{g}", name=f"taub{g}")
                    mask = pp.tile([128, 8], mybir.dt.float32, tag=f"mask{g}", name=f"mask{g}")
                    cnt = pp.tile([128, 1], mybir.dt.float32, tag=f"cnt{g}", name=f"cnt{g}")
                    Pcol = pp.tile([128, 1], mybir.dt.float32, tag=f"Pcol{g}", name=f"Pcol{g}")
                    pos = pp.tile([128, 8], mybir.dt.float32, tag=f"pos{g}", name=f"pos{g}")
                    oh = pp.tile([128, 128], mybir.dt.float32, tag=f"oh{g}", name=f"oh{g}")
                    dsb = pp.tile([128, 3], mybir.dt.float32, tag=f"dsb{g}", name=f"dsb{g}")
                    vfix = pp.tile([128, 1], mybir.dt.float32, tag=f"vfix{g}", name=f"vfix{g}")
                    vr = pp.tile([1, 128], mybir.dt.float32, tag=f"vr{g}", name=f"vr{g}")
                    Cm = pp.tile([128, 128], mybir.dt.float32, tag=f"Cm{g}", name=f"Cm{g}")
                    Rk = pp.tile([128, 1], mybir.dt.float32, tag=f"Rk{g}", name=f"Rk{g}")
                    ohp = pp.tile([128, 128], mybir.dt.float32, tag=f"ohp{g}", name=f"ohp{g}")
                    ps_a = psp.tile([128, 128], mybir.dt.float32, tag="psa", name=f"psa{g}")
                    ps_b = psp.tile([128, 512], mybir.dt.float32, tag="psb", name=f"psb{g}")
                    ps_c = psp.tile([128, 8], mybir.dt.float32, tag="psc", name=f"psc{g}")

                    cvb = cv[:, 8 * b:8 * b + 8]
                    nc.tensor.matmul(out=ps_a[0:1, 0:128], lhsT=cvb[:, 0:1], rhs=ID,
                                     start=True, stop=True)
                    nc.scalar.copy(out=mrow[:], in_=ps_a[0:1, 0:128])
                    nc.tensor.matmul(out=ps_a[:, 0:128], lhsT=ONESROW, rhs=mrow[:],
                                     start=True, stop=True)
                    for r in range(8):
                        nc.vector.tensor_scalar(
                            out=Call[:, 128 * r:128 * r + 128], in0=ps_a[:, 0:128],
                            scalar1=cvb[:, r:r + 1], scalar2=None, op0=AL.is_lt)
                    nc.tensor.matmul(out=ps_b[0:1, 0:512], lhsT=ONESCOL,
                                     rhs=Call[:, 0:512], start=True, stop=False)
                    nc.tensor.matmul(out=ps_b[0:1, 0:512], lhsT=ONESCOL,
                                     rhs=Call[:, 512:1024], start=False, stop=True)
                    s2 = bass.AP(ps_b[:].tensor, ps_b[:].offset,
                                 [[512, 1], [1, 128], [128, 4]])
                    nc.vector.tensor_reduce(out=cntrow[:], in_=s2,
                                            axis=mybir.AxisListType.X, op=AL.add)
                    nc.vector.tensor_scalar(out=tsel[:], in0=cntrow[:], scalar1=128.5,
                                            scalar2=1e30, op0=AL.is_gt, op1=AL.mult)
                    nc.vector.tensor_add(out=tsel[:], in0=tsel[:], in1=mrow[:])
                    nc.vector.tensor_reduce(out=tmin[:], in_=tsel[:],
                                            axis=mybir.AxisListType.X, op=AL.min)
                    nc.tensor.matmul(out=ps_a[:, 0:1], lhsT=ONESROW, rhs=tmin[:],
                                     start=True, stop=True)
                    nc.scalar.copy(out=taub[:], in_=ps_a[:, 0:1])
                    nc.vector.tensor_scalar(out=mask[:], in0=cvb, scalar1=taub[:],
                                            scalar2=None, op0=AL.is_gt)
                    nc.vector.tensor_reduce(out=cnt[:], in_=mask[:],
                                            axis=mybir.AxisListType.X, op=AL.add)
                    nc.tensor.matmul(out=ps_a[:, 0:1], lhsT=LT, rhs=cnt[:],
                                     start=True, stop=True)
                    nc.scalar.copy(out=Pcol[:], in_=ps_a[:, 0:1])
                    nc.vector.tensor_scalar(out=pos[:], in0=mask[:], scalar1=-1000.0,
                                            scalar2=1000.0, op0=AL.mult, op1=AL.add)
                    nc.vector.tensor_add(out=pos[:], in0=pos[:], in1=R8)
                    nc.vector.tensor_scalar(out=pos[:], in0=pos[:], scalar1=Pcol[:],
                                            scalar2=None, op0=AL.add)
                    for r in range(8):
                        nc.vector.tensor_scalar(out=oh[:], in0=IR,
                                                scalar1=pos[:, r:r + 1], scalar2=None,
                                                op0=AL.is_equal)
                        nc.tensor.matmul(out=ps_c[:, 0:3], lhsT=oh[:],
                                         rhs=pairs[:, 24 * b + 3 * r:24 * b + 3 * r + 3],
                                         start=(r == 0), stop=(r == 7))
                    nc.scalar.copy(out=dsb[:], in_=ps_c[:, 0:3])
                    nc.vector.tensor_scalar(out=vfix[:], in0=dsb[:, 2:3], scalar1=1e30,
                                            scalar2=-1e30, op0=AL.mult, op1=AL.add)
                    nc.vector.tensor_add(out=vfix[:], in0=vfix[:], in1=dsb[:, 0:1])
                    nc.tensor.matmul(out=ps_a[0:1, 0:128], lhsT=vfix[:], rhs=ID,
                                     start=True, stop=True)
                    nc.scalar.copy(out=vr[:], in_=ps_a[0:1, 0:128])
                    nc.tensor.matmul(out=ps_a[:, 0:128], lhsT=ONESROW, rhs=vr[:],
                                     start=True, stop=True)
                    nc.vector.tensor_scalar(out=Cm[:], in0=ps_a[:, 0:128],
                                            scalar1=vfix[:], scalar2=None, op0=AL.is_gt)
                    # tie-break by pixel index ascending: rank += (v==v_i) & (pix_j < pix_i)
                    nc.vector.tensor_scalar(out=oh[:], in0=ps_a[:, 0:128],
                                            scalar1=vfix[:], scalar2=None, op0=AL.is_equal)
                    nc.tensor.matmul(out=ps_b[0:1, 0:128], lhsT=dsb[:, 1:2], rhs=ID,
                                     start=True, stop=True)
                    nc.scalar.copy(out=vr[:], in_=ps_b[0:1, 0:128])
                    nc.tensor.matmul(out=ps_b[:, 0:128], lhsT=ONESROW, rhs=vr[:],
                                     start=True, stop=True)
                    nc.vector.tensor_scalar(out=ps_b[:, 0:128], in0=ps_b[:, 0:128],
                                            scalar1=dsb[:, 1:2], scalar2=None, op0=AL.is_lt)
                    nc.vector.tensor_mul(out=oh[:], in0=oh[:], in1=ps_b[:, 0:128])
                    nc.vector.tensor_add(out=Cm[:], in0=Cm[:], in1=oh[:])
                    nc.vector.tensor_reduce(out=Rk[:], in_=Cm[:],
                                            axis=mybir.AxisListType.X, op=AL.add)
                    nc.vector.tensor_scalar(out=ohp[:], in0=IR, scalar1=Rk[:],
                                            scalar2=None, op0=AL.is_equal)
                    nc.tensor.matmul(out=ps_c[:, 4:6], lhsT=ohp[:], rhs=dsb[:, 0:2],
                                     start=True, stop=True)
                    nc.scalar.copy(out=srt[:, 2 * b:2 * b + 2], in_=ps_c[:, 4:6])

            # ---- gather loc, decode, NMS ----
            with nc.named_scope("decode"):
                valc = po.tile([128, 4], mybir.dt.float32, tag="valc")
                pixc = po.tile([128, 4], mybir.dt.float32, tag="pixc")
                offs = po.tile([128, 4], mybir.dt.float32, tag="offs")
                offi = po.tile([128, 4], mybir.dt.int32, tag="offi")
                tg = po.tile([128, 16], mybir.dt.float32, tag="tg")
                score = po.tile([128, 4], mybir.dt.float32, tag="score")
                xpix = po.tile([128, 4], mybir.dt.float32, tag="xpix")
                ypix = po.tile([128, 4], mybir.dt.float32, tag="ypix")
                bx = po.tile([128, 4], mybir.dt.float32, tag="bx")
                by = po.tile([128, 4], mybir.dt.float32, tag="by")
                h2 = po.tile([128, 4], mybir.dt.float32, tag="h2")
                v2 = po.tile([128, 4], mybir.dt.float32, tag="v2")
                x1 = po.tile([128, 4], mybir.dt.float32, tag="x1")
                x2 = po.tile([128, 4], mybir.dt.float32, tag="x2")
                y1 = po.tile([128, 4], mybir.dt.float32, tag="y1")
                y2 = po.tile([128, 4], mybir.dt.float32, tag="y2")
                area = po.tile([128, 4], mybir.dt.float32, tag="area")
                dpack = po.tile([128, 24], mybir.dt.float32, tag="dpack")

                vsrc = bass.AP(srt[:].tensor, srt[:].offset, [srt[:].ap[0], [2, 4]])
                qsrc = bass.AP(srt[:].tensor, srt[:].offset + 1, [srt[:].ap[0], [2, 4]])
                nc.vector.tensor_copy(out=valc[:], in_=vsrc)
                nc.vector.tensor_copy(out=pixc[:], in_=qsrc)
                nc.scalar.activation(out=score[:], in_=valc[:], func=AF.Sigmoid)
                nc.vector.tensor_add(out=offs[:], in0=pixc[:], in1=BN)
                nc.vector.tensor_copy(out=offi[:], in_=offs[:])
                for b in range(BPC):
                    nc.gpsimd.indirect_dma_start(
                        out=tg[:, 4 * b:4 * b + 4], out_offset=None, in_=loc_in.ap(),
                        in_offset=bass.IndirectOffsetOnAxis(ap=offi[:, b:b + 1], axis=1))
                # y = trunc(pix/480) (exact for ints < 2^24); x = pix - 480*y
                nc.vector.tensor_scalar(out=ypix[:], in0=pixc[:], scalar1=1.0 / 480.0,
                                        scalar2=-0.4995, op0=AL.mult, op1=AL.add)
                nc.vector.tensor_copy(out=offi[:], in_=ypix[:])
                nc.vector.tensor_copy(out=ypix[:], in_=offi[:])
                nc.vector.scalar_tensor_tensor(out=xpix[:], in0=ypix[:], scalar=-480.0,
                                               in1=pixc[:], op0=AL.mult, op1=AL.add)
                nc.vector.tensor_scalar(out=xpix[:], in0=xpix[:], scalar1=4.0,
                                        scalar2=1.5, op0=AL.mult, op1=AL.add)
                nc.vector.tensor_scalar(out=ypix[:], in0=ypix[:], scalar1=4.0,
                                        scalar2=1.5, op0=AL.mult, op1=AL.add)
                t0 = bass.AP(tg[:].tensor, tg[:].offset + 0, [tg[:].ap[0], [4, 4]])
                t1 = bass.AP(tg[:].tensor, tg[:].offset + 1, [tg[:].ap[0], [4, 4]])
                t2 = bass.AP(tg[:].tensor, tg[:].offset + 2, [tg[:].ap[0], [4, 4]])
                t3 = bass.AP(tg[:].tensor, tg[:].offset + 3, [tg[:].ap[0], [4, 4]])
                nc.vector.tensor_scalar(out=bx[:], in0=t0, scalar1=float(W * 4),
                                        scalar2=None, op0=AL.mult)
                nc.vector.tensor_add(out=bx[:], in0=bx[:], in1=xpix[:])
                nc.vector.tensor_scalar(out=by[:], in0=t1, scalar1=float(H * 4),
                                        scalar2=None, op0=AL.mult)
                nc.vector.tensor_add(out=by[:], in0=by[:], in1=ypix[:])
                nc.vector.tensor_scalar(out=h2[:], in0=t2, scalar1=float(W * 4),
                                        scalar2=0.5, op0=AL.mult, op1=AL.mult)
                nc.vector.tensor_scalar(out=v2[:], in0=t3, scalar1=float(H * 4),
                                        scalar2=0.5, op0=AL.mult, op1=AL.mult)
                nc.vector.tensor_sub(out=x1[:], in0=bx[:], in1=h2[:])
                nc.vector.tensor_add(out=x2[:], in0=bx[:], in1=h2[:])
                nc.vector.tensor_sub(out=y1[:], in0=by[:], in1=v2[:])
                nc.vector.tensor_add(out=y2[:], in0=by[:], in1=v2[:])
                nc.vector.tensor_sub(out=area[:], in0=x2[:], in1=x1[:])
                nc.vector.tensor_sub(out=dpack[:, 20:24], in0=y2[:], in1=y1[:])
                nc.vector.tensor_mul(out=area[:], in0=area[:], in1=dpack[:, 20:24])
                # pack det columns [x1 y1 x2 y2 score] per image
                for qi, q in enumerate((x1, y1, x2, y2, score)):
                    dst = bass.AP(dpack[:].tensor, dpack[:].offset + qi, [dpack[:].ap[0], [5, 4]])
                    nc.vector.tensor_copy(out=dst, in_=q[:])
                for b in range(BPC):
                    nc.sync.dma_start(out=det_out.ap()[b], in_=dpack[0:NK, 5 * b:5 * b + 5])

            with nc.named_scope("nms"):
                rows = po.tile([1, 5 * 128 * BPC], mybir.dt.float32, tag="rows")
                XB = po.tile([100, 400], mybir.dt.float32, tag="XB")
                keepc = po.tile([100, BPC], mybir.dt.float32, tag="keepc")
                ps_r = psp.tile([1, 512], mybir.dt.float32, tag="ps_r")
                ps_B = psp.tile([100, 400], mybir.dt.float32, tag="ps_B")
                ps_j = psp.tile([100, 4], mybir.dt.float32, tag="ps_j")
                wq = po.tile([100, 400], mybir.dt.float32, tag="wq")
                hq = po.tile([100, 400], mybir.dt.float32, tag="hq")
                inter = po.tile([100, 400], mybir.dt.float32, tag="inter")
                uni = po.tile([100, 400], mybir.dt.float32, tag="uni")
                Smat = po.tile([100, 400], mybir.dt.float32, tag="Smat")

                # rows: for q in (x1,y1,x2,y2,area): per image transpose [128,1]->[1,128]
                for qi, q in enumerate((x1, y1, x2, y2, area)):
                    for b in range(BPC):
                        nc.tensor.matmul(out=ps_r[0:1, 128 * b:128 * b + 128],
                                         lhsT=q[:, b:b + 1], rhs=ID,
                                         start=True, stop=True)
                    nc.scalar.copy(out=bass.AP(rows[:].tensor, rows[:].offset + 512 * qi,
                                               [rows[:].ap[0], [1, 512]]),
                                   in_=ps_r[0:1, 0:512])
                # per quantity broadcast [100, 400] (cols 100b+j = q_b[j])
                qsl = []
                for qi, (q, op) in enumerate(((x1, AL.max), (y1, AL.max),
                                              (x2, AL.min), (y2, AL.min))):
                    rsel = bass.AP(rows[:].tensor, rows[:].offset + 512 * qi,
                                   [[5 * 128 * BPC, 1], [128, 4], [1, 100]])
                    nc.tensor.matmul(out=ps_B[:], lhsT=cst[0:1, OFF_ONES:OFF_ONES + 100],
                                     rhs=rsel, start=True, stop=True)
                    dst = (wq, hq, inter, uni)[qi]
                    for b in range(BPC):
                        nc.vector.tensor_scalar(out=dst[:, 100 * b:100 * b + 100],
                                                in0=ps_B[:, 100 * b:100 * b + 100],
                                                scalar1=q[0:100, b:b + 1], scalar2=None,
                                                op0=op)
                # w = xx2 - xx1 (relu), h = yy2 - yy1 (relu), inter = w*h
                nc.vector.tensor_sub(out=wq[:], in0=inter[:], in1=wq[:])
                nc.vector.tensor_scalar(out=wq[:], in0=wq[:], scalar1=0.0,
                                        scalar2=None, op0=AL.max)
                nc.vector.tensor_sub(out=hq[:], in0=uni[:], in1=hq[:])
                nc.vector.tensor_scalar(out=hq[:], in0=hq[:], scalar1=0.0,
                                        scalar2=None, op0=AL.max)
                nc.vector.tensor_mul(out=inter[:], in0=wq[:], in1=hq[:])
                # union = area_i + area_j - inter + 1e-9
                rsel = bass.AP(rows[:].tensor, rows[:].offset + 512 * 4,
                               [[5 * 128 * BPC, 1], [128, 4], [1, 100]])
                nc.tensor.matmul(out=ps_B[:], lhsT=cst[0:1, OFF_ONES:OFF_ONES + 100],
                                 rhs=rsel, start=True, stop=True)
                for b in range(BPC):
                    nc.vector.scalar_tensor_tensor(
                        out=uni[:, 100 * b:100 * b + 100],
                        in0=ps_B[:, 100 * b:100 * b + 100],
                        scalar=area[0:100, b:b + 1],
                        in1=inter[:, 100 * b:100 * b + 100],
                        op0=AL.add, op1=AL.subtract)
                nc.vector.tensor_scalar(out=uni[:], in0=uni[:], scalar1=1e-9,
                                        scalar2=None, op0=AL.add)
                nc.vector.reciprocal(out=uni[:], in_=uni[:])
                nc.vector.tensor_mul(out=inter[:], in0=inter[:], in1=uni[:])
                # S = (iou > 0.7) & (j > i); LT tiled x4
                lt4 = bass.AP(cst[:].tensor, cst[:].offset + OFF_LT,
                              [[CW, 100], [0, 4], [1, 100]])
                nc.vector.scalar_tensor_tensor(out=Smat[:], in0=inter[:], scalar=0.7,
                                               in1=lt4, op0=AL.is_gt, op1=AL.mult)
                # Jacobi x2: keep = 1; keep = !(S^T keep > 0.5)
                nc.vector.memset(keepc[:], 1.0)
                for it in range(2):
                    for b in range(BPC):
                        nc.tensor.matmul(out=ps_j[:, b:b + 1],
                                         lhsT=Smat[:, 100 * b:100 * b + 100],
                                         rhs=keepc[:, b:b + 1], start=True, stop=True)
                    nc.vector.tensor_scalar(out=keepc[:], in0=ps_j[:], scalar1=0.5,
                                            scalar2=None, op0=AL.is_lt)
                for b in range(BPC):
                    nc.sync.dma_start(out=keep_out.ap()[b], in_=keepc[:, b:b + 1])
    nc.compile()
    return nc


def _get_nc():
    if "nc" not in _CACHE:
        _install_hook()
        _CACHE["nc"] = build()
    return _CACHE["nc"]


def kernel(cls_map: np.ndarray, loc_map: np.ndarray):
    """Full inputs [32,2,270,480] f32 + [32,4,270,480] f32 ->
    (detections [32,100,5] f32, keep [32,100] bool)."""
    from concourse import bass_utils

    cls_map = np.asarray(cls_map, dtype=np.float32)
    loc_map = np.asarray(loc_map, dtype=np.float32)

    cstv = make_consts()
    in_maps = []
    for core in range(NCORES):
        sl = slice(core * BPC, (core + 1) * BPC)
        cls = cls_map[sl].reshape(BPC, 2, N)
        arr = np.empty((BPC, 2, 128 * F), np.float32)
        arr[:, 0, :N] = cls[:, 0]
        arr[:, 1, :N] = cls[:, 1]
        arr[:, 0, N:] = 1e30
        arr[:, 1, N:] = -1e30
        # [BPC, 2, 128, F] -> [128, 2, BPC, F]
        clsP = np.ascontiguousarray(
            arr.reshape(BPC, 2, 128, F).transpose(2, 1, 0, 3))
        locT = np.ascontiguousarray(
            loc_map[sl].reshape(BPC, 4, N).transpose(0, 2, 1))
        in_maps.append({"cls": clsP, "loc": locT, "cst": cstv})

    res = bass_utils.run_bass_kernel_spmd(
        _get_nc(), in_maps, core_ids=list(range(NCORES)))

    det = np.empty((B, NK, 5), np.float32)
    keep = np.empty((B, NK), bool)
    for core in range(NCORES):
        det[core * BPC:(core + 1) * BPC] = res.results[core]["det"]
        keep[core * BPC:(core + 1) * BPC] = res.results[core]["keep"] > 0.5
    return det, keep
